# revision 30
# baseline (speedup 1.0000x reference)
"""Trainium2 Bass kernel for batched per-frequency steering-matrix application.

Computes Y[b,t,k,n] = sum_m X[b,t,k,m] * (U_real + i*U_imag)[pid[b],k,m,n]
as complex64, distributed data-parallel over batch across 8 NeuronCores.

Device strategy per core (2 batch samples), engine-balanced:
  - Transposed matmul orientation: block-diagonal steering weights are the
    STATIONARY operand ([128 K=(ks,m), 128 M=(ks',n,c)] bf16 halves) and X
    streams as the MOVING operand (fp8, t columns) -> PSUM [128, 400].
    That is the minimum possible moving-column count (output elements /
    128 partitions), and LDWEIGHTS fully overlaps the matmuls.
  - Block-diag expansion: one broadcast-multiply per PAIR of 13-group
    chunks (GpSimd has ~6.5 us FIXED cost per instruction, so few big ops)
    -- bd = dense_ud * mask writes the zeros itself (no memset, no
    scatter DMAs).  GpSimd runs pairs 1-4; Vector covers pair 0 in three
    pieces so the first matmul starts early.
  - PSUM->SBUF int8 quantizing conversions are the scarcest resource
    (TRN2 PSUM f32 sources run at 1x: DVE ~1.04 ns/col, ACT ~0.83 ns/col):
    two 400-col units per [128, 1024] f32 PSUM tile (2 banks) converted in
    one 800-col run.  The 4-deep tile rotation eliminates the
    conv->refill serialization that a 2-deep rotation of bigger tiles
    suffers.  Vector and Scalar do nothing else, alternating with 9 megas
    swapped to ACT for its faster clock (56/74).
  - Matmuls are split at PSUM 2 KB bank boundaries -- a matmul whose
    output crosses a bank boundary corrupts the columns past the
    boundary on TRN2 hardware (CoreSim does not model this).
  - Output is int8: host folds a 127/CLIP scale into U, hardware does RNE
    + saturating f32->int8 on the PSUM evacuation, host dequantizes.
  - DMA dispatches cost ~0.6-0.8 us of sequencer time each, so the count
    is a first-class budget: ~44 total, ordered so output head-of-line
    waits never starve the 3-chunk-deep x prefetch.
"""

import sys

for _p in ("/opt/trn_rl_repo", "/root/.axon_site/_ro/trn_rl_repo"):
    if _p not in sys.path:
        sys.path.append(_p)

import numpy as np
import ml_dtypes


def _install_ntff_hook_shim():
    """The image's antenv lacks axon_hooks; synthesize it so trace=True can
    capture NTFF profiles via /opt/axon/libaxon_pjrt.so."""
    try:
        import antenv.axon_hooks  # noqa: F401
        return
    except ImportError:
        pass
    import types
    import contextlib
    import ctypes

    mod = types.ModuleType("antenv.axon_hooks")
    mod._hook = None

    def set_axon_ntff_profile_hook(h):
        mod._hook = h

    def get_axon_ntff_profile_hook():
        return mod._hook

    mod.set_axon_ntff_profile_hook = set_axon_ntff_profile_hook
    mod.get_axon_ntff_profile_hook = get_axon_ntff_profile_hook
    sys.modules["antenv.axon_hooks"] = mod
    try:
        import antenv

        antenv.axon_hooks = mod
    except ImportError:
        pass

    so_path = "/opt/axon/libaxon_pjrt.so"
    try:
        lib = ctypes.CDLL(so_path)
        if not hasattr(lib, "axon_start_nrt_profile"):
            return
        lib.axon_start_nrt_profile.argtypes = [
            ctypes.POINTER(ctypes.c_int64),
            ctypes.c_size_t,
        ]
        lib.axon_start_nrt_profile.restype = ctypes.c_int64
        lib.axon_stop_nrt_profile.argtypes = [ctypes.c_char_p]
        lib.axon_stop_nrt_profile.restype = ctypes.c_int64
    except OSError:
        return

    @contextlib.contextmanager
    def _hook(output_dir, device_ids):
        import jax

        jax.devices()
        if device_ids:
            ids = (ctypes.c_int64 * len(device_ids))(*device_ids)
            rc = lib.axon_start_nrt_profile(ids, len(device_ids))
        else:
            rc = lib.axon_start_nrt_profile(None, 0)
        if rc != 0:
            raise RuntimeError(f"axon_start_nrt_profile rc={rc}")
        try:
            yield
        finally:
            n = lib.axon_stop_nrt_profile(str(output_dir).encode())
            print(f"ntff profile: {n} file(s) written to {output_dir}", file=sys.stderr)

    mod._hook = _hook


_install_ntff_hook_shim()

# ---- problem constants (hardcoded per spec) ----
NDOA, B, T, NBIN, NMIC = 36, 16, 400, 513, 16
NCORES = 8
BLOC = B // NCORES        # 2 batch samples per core
NG = 65                   # groups of 8 bins; 513 padded to 520
NBIN_PAD = NG * 8
NC2 = 2 * NMIC            # 32 interleaved (n, c) per bin
GPC = 13                  # bin groups per load chunk (5 chunks per b)
NCH = NG // GPC
NCHNK = BLOC * NCH        # 10 chunks total
NUPB = 2 * NG             # 130 (g, half) units per batch sample
MEGA = 2                  # units per PSUM tile (2 banks of 512 f32)
STU = 20                  # units per output staging tile (last one holds 10)
# bank-boundary splits per unit slot within the 1024-col PSUM tile
SPLITS = {0: ((0, 400),), 1: ((0, 112), (112, 400))}
# conversion megas swapped from Vector to the faster Scalar clock (56/74)
ASW = frozenset((21, 49, 77, 91, 105, 119))

# int8 output quantization: saturate at |Y| = CLIP (|Y| has sigma ~4; the
# handful of clipped outliers contribute less error than a coarser step)
CLIP = 18.0
QSCALE = 127.0 / CLIP

_cache = {}


def _build(trace=False):
    """Build the Bass graph (one SPMD program, same for all cores)."""
    import concourse.bass as bass
    import concourse.mybir as mybir
    import concourse.tile as tile
    from concourse import bacc

    nc = bacc.Bacc(None, target_bir_lowering=False)

    x = nc.declare_dram_parameter("x", [BLOC, 128, NG, T], mybir.dt.float8e3, isOutput=False)
    # dense gathered U[pid], host-scaled by QSCALE: [b, p=(ks,m), g, (n,c)]
    ud = nc.declare_dram_parameter(
        "ud", [BLOC, 128, NG, NC2], mybir.dt.bfloat16, isOutput=False
    )
    # block-diag selection mask: mk[p, 0, ks', j] = (p//16 == ks')
    mk = nc.declare_dram_parameter("mk", [128, 1, 8, NC2], mybir.dt.bfloat16, isOutput=False)
    # dense int8 output: per b, unit blocks of [p=(ks_rel,n,c), u_local, t]
    out = nc.declare_dram_parameter(
        "out", [BLOC, NUPB * 128 * T], mybir.dt.int8, isOutput=True
    )

    with tile.TileContext(nc) as tc:
        with (
            tc.tile_pool(name="xp", bufs=7) as xp,
            tc.tile_pool(name="udp", bufs=5) as udp,
            tc.tile_pool(name="bdp", bufs=3) as bdp,
            tc.tile_pool(name="mkp", bufs=1) as mkp,
            tc.tile_pool(name="stp", bufs=6) as stp,
            tc.tile_pool(name="psum", bufs=4, space="PSUM") as psum,
        ):
            # chunk-PAIR granularity for the weight path (one big op each)
            uds = [
                udp.tile([128, 2 * GPC, 1, NC2], mybir.dt.bfloat16, tag="ud", name=f"ud{i}")
                for i in range(5)
            ]
            bds = [
                bdp.tile([128, 2 * GPC, 8, NC2], mybir.dt.bfloat16, tag="bd", name=f"bd{r}")
                for r in range(3)
            ]
            mks = mkp.tile([128, 1, 8, NC2], mybir.dt.bfloat16, tag="mk", name="mk")
            xts = {}

            def stage_pair(p):
                # pairs (0,1),(2,3) are b0-contiguous; (4,5) straddles b0/b1;
                # (6,7),(8,9) are b1-contiguous
                g0 = (2 * p) % NCH * GPC
                if p == 2:
                    nc.sync.dma_start(uds[p][:, :GPC], ud[0, :, 52:65])
                    nc.sync.dma_start(uds[p][:, GPC:], ud[1, :, 0:13])
                else:
                    b = (2 * p) // NCH
                    nc.sync.dma_start(uds[p][:], ud[b, :, g0 : g0 + 2 * GPC])

            def expand(p, gs, ge, eng):
                # bd = broadcast(dense ud) * broadcast(mask): the mask zeros
                # write the off-diagonal, so no memset and no scatter DMAs
                shp = [128, ge - gs, 8, NC2]
                eng.tensor_mul(
                    bds[p % 3][:, gs:ge],
                    uds[p][:, gs:ge].broadcast_to(shp),
                    mks.broadcast_to(shp),
                )

            def load_x(c, first=False):
                b, ci = divmod(c, NCH)
                g0 = ci * GPC
                xt = xp.tile([128, GPC, T], mybir.dt.float8e3, tag="x")
                if first:
                    # kernel-start critical path: fan the first chunk over
                    # three DMA queues so the first matmul starts early
                    nc.sync.dma_start(xt[:, 0:2], x[b, :, g0 : g0 + 2])
                    nc.scalar.dma_start(xt[:, 2:7], x[b, :, g0 + 2 : g0 + 7])
                    nc.gpsimd.dma_start(xt[:, 7:], x[b, :, g0 + 7 : g0 + GPC])
                else:
                    nc.sync.dma_start(xt[:, :7], x[b, :, g0 : g0 + 7])
                    nc.sync.dma_start(xt[:, 7:], x[b, :, g0 + 7 : g0 + GPC])
                xts[c] = xt

            # pre-roll: mask + first pair's inputs first (they gate the
            # first expansion + matmuls), then a 3-chunk-deep x prefetch.
            # Vector expands pair 0 in three pieces (the first 2 groups gate
            # the very first matmuls); GpSimd takes pairs 1-4
            nc.sync.dma_start(mks[:], mk[:])
            stage_pair(0)
            load_x(0, first=True)
            expand(0, 0, 2, nc.vector)
            expand(0, 2, GPC, nc.gpsimd)
            stage_pair(1)
            stage_pair(2)
            load_x(1)
            load_x(2)
            expand(0, GPC, 2 * GPC, nc.gpsimd)
            expand(1, 0, 2 * GPC, nc.gpsimd)
            expand(2, 0, 2 * GPC, nc.gpsimd)
            load_x(3)

            nmega = 0

            def use_vector(i):
                return i % 2 == 1 and i not in ASW

            for b in range(BLOC):
                for u in range(NUPB):
                    g, h = divmod(u, 2)
                    c = b * NCH + g // GPC
                    gl = g % GPC
                    if h == 0 and gl == 0 and c >= 1:
                        # chunk boundary: keep the x prefetch 3 deep; at even
                        # boundaries stage + expand the pair-after-next
                        # (WAR on its bd tile resolves exactly now)
                        if c + 3 < NCHNK:
                            load_x(c + 3)
                        if c % 2 == 0 and c // 2 + 2 < 5:
                            p = c // 2 + 2
                            stage_pair(p)
                            expand(p, 0, 2 * GPC, nc.gpsimd)
                    k = u % MEGA
                    if k == 0:
                        ps = psum.tile([128, MEGA * 512], mybir.dt.float32, tag="ps")
                    if u % STU == 0:
                        w = min(STU, NUPB - u)
                        st = stp.tile([128, w, T], mybir.dt.int8, tag="st")
                        base = u * 128 * T
                        dstv = out[b, base : base + 128 * w * T].rearrange(
                            "(p q) -> p q", q=w * T
                        )
                    bdg = bds[(c // 2) % 3][:, (c % 2) * GPC + gl].rearrange(
                        "p a j -> p (a j)"
                    )
                    lhsT = bdg[:, 128 * h : 128 * h + 128]
                    for t0, t1 in SPLITS[k]:
                        nc.tensor.matmul(
                            ps[:, k * T + t0 : k * T + t1],
                            lhsT,
                            xts[c][:, gl, t0:t1],
                            start=True,
                            stop=True,
                        )
                    if k == MEGA - 1:
                        # quantizing PSUM->SBUF evacuation of both units in
                        # one 800-col run (hw does RNE + saturation)
                        jm = (u % STU) // MEGA
                        dst = st[:, jm * MEGA : jm * MEGA + MEGA]
                        src = ps[:, : MEGA * T]
                        if use_vector(nmega):
                            nc.vector.tensor_copy(dst, src)
                        else:
                            nc.scalar.copy(dst, src)
                        nmega += 1
                    if u % STU == STU - 1 or u == NUPB - 1:
                        nc.sync.dma_start(dstv, st[:])
    nc.compile()
    return nc


def _get_nc():
    if "nc" not in _cache:
        _cache["nc"] = _build()
    return _cache["nc"]


def _host_prep(X, pid, U_real, U_imag):
    X = np.asarray(X, np.float32)
    pid = np.asarray(pid).astype(np.int64)
    U_real = np.asarray(U_real, np.float32)
    U_imag = np.asarray(U_imag, np.float32)

    # gather + stack real/imag, fold in quantization scale: [B, NBIN, M, N, 2]
    Ug = np.stack([U_real[pid], U_imag[pid]], axis=-1) * QSCALE
    Ug_p = np.zeros((B, NBIN_PAD, NMIC, NMIC, 2), np.float32)
    Ug_p[:, :NBIN] = Ug
    # dense source, partition-major: [b, p=(ks,m), g, (n,c)]
    Udr = Ug_p.reshape(B, NG, 8, NMIC, NMIC, 2).transpose(0, 2, 3, 1, 4, 5)
    Ud = np.ascontiguousarray(
        Udr.reshape(B, 128, NG, NC2)
    ).astype(ml_dtypes.bfloat16)

    # X: [b,t,k,m] -> [b,k,m,t] -> pad -> [b, p=(ks,m), g, t]
    Xt = X.transpose(0, 2, 3, 1)
    Xp_ = np.zeros((B, NBIN_PAD, NMIC, T), np.float32)
    Xp_[:, :NBIN] = Xt
    Xp_ = Xp_.reshape(B, NG, 8, NMIC, T).transpose(0, 2, 3, 1, 4)
    Xp = np.ascontiguousarray(Xp_.reshape(B, 128, NG, T)).astype(ml_dtypes.float8_e3m4)
    return Xp, Ud


def _make_mask():
    m = (np.arange(128)[:, None] // NMIC == np.arange(8)[None, :]).astype(np.float32)
    return np.ascontiguousarray(
        np.broadcast_to(m[:, None, :, None], (128, 1, 8, NC2))
    ).astype(ml_dtypes.bfloat16)


def _unshuffle(full):
    """[B, NUPB*128*T] int8 -> complex64 [B, T, NBIN, NMIC].

    Device layout per b: staging blocks of [p=(ks_rel,n,c), u_local, t]
    (6 blocks of 20 units + 1 of 10), u = 2g + h, bin = g*8 + h*4 + ks_rel."""
    nmain = 6 * 128 * STU * T
    main = full[:, :nmain].reshape(B, 6, 128, STU, T).transpose(0, 1, 3, 2, 4)
    tail = full[:, nmain:].reshape(B, 1, 128, 10, T).transpose(0, 1, 3, 2, 4)
    a = np.concatenate(
        [main.reshape(B, 6 * STU, 128, T), tail.reshape(B, 10, 128, T)], axis=1
    ).astype(np.float32)
    a *= CLIP / 127.0
    # dims (b, (g,h), (ks_rel,n,c), t) -> (b, t, g, h, ks_rel, n, c)
    a = a.reshape(B, NG, 2, 4, NMIC, 2, T)
    a = np.ascontiguousarray(a.transpose(0, 6, 1, 2, 3, 4, 5))
    a = a.reshape(B, T, NBIN_PAD, NMIC, 2)
    c = a.view(np.complex64)[..., 0]
    return np.ascontiguousarray(c[:, :, :NBIN])


def _run(in_maps, trace=False):
    from concourse.bass_utils import run_bass_kernel_spmd

    nc = _get_nc()
    res = run_bass_kernel_spmd(nc, in_maps, core_ids=list(range(NCORES)), trace=trace)
    return res


def kernel(X, pid, U_real, U_imag, _trace=False, _return_results=False):
    Xp, Ud = _host_prep(X, pid, U_real, U_imag)
    mkc = _make_mask()
    in_maps = [
        {
            "x": np.ascontiguousarray(Xp[i * BLOC : (i + 1) * BLOC]),
            "ud": np.ascontiguousarray(Ud[i * BLOC : (i + 1) * BLOC]),
            "mk": mkc,
        }
        for i in range(NCORES)
    ]
    res = _run(in_maps, trace=_trace)
    full = np.concatenate([r["out"] for r in res.results], axis=0)
    out = _unshuffle(full)
    if _return_results:
        return out, res
    return out
